# revision 51
# baseline (speedup 1.0000x reference)
"""EnhancedGovernanceAttention Trainium2 kernel (8 NeuronCores, SPMD).

Sharding: core c owns heads {2c, 2c+1} for BOTH batches (policy_mask is
per-head and batch-shared, so each policy slice is loaded once per core).
Each core computes its heads' attention and a row-parallel partial of the
Wo projection; the host sums the 8 partials and adds bo.

Math notes (vs the jax reference):
 - softmax max-subtraction is dropped: scores ~ N(0,1) + bias in [0,0.2],
   so exp() cannot overflow in fp32; softmax is shift-invariant.
 - log1p memory bias: softmax(s + log(w)) == (w * exp(s)) / sum(w * exp(s))
   with w = 1 + GS*mw + 1e-8, so w is folded into V rows and into the
   denominator matmul -- no per-score log bias needed.
 - causal mask: only lower-triangle k-tiles are computed; the intra-tile
   diagonal mask is baked into the (bf16) policy bias as -40.
 - scores are computed TRANSPOSED ([k, q]) so the PV matmul directly
   yields attn^T, which is the lhsT the output projection needs.
 - x^T is produced by bf16 hi/lo DMA-transposes + one DVE add (exact to
   ~2^-16 relative), avoiding PE-transpose traffic for x.
 - matmuls run in float32r (~1.8e-4 quantization, 4x faster than fp32).
"""

import numpy as np
import ml_dtypes
from contextlib import ExitStack

import concourse.bass as bass
import concourse.tile as tile
from concourse import bacc, mybir
from concourse.bass_utils import run_bass_kernel_spmd
from concourse.masks import make_identity

B, S, D, H, HD = 2, 2048, 2048, 16, 128
GS = 0.1
ROPE_BASE = 10000.0
NCORES = 8
HPC = H // NCORES          # heads per core = 2
SCALE = float(HD) ** -0.5
DT = D // 128              # 16 d-tiles
ST = S // 128              # 16 s-tiles (also k-tiles)
QB = 512                   # q-block width (phase B)
NQB = S // QB              # 4 q-blocks
SB = 256                   # s-block width (phase A panels)
NSB = S // SB              # 8 s-blocks
MASK_NEG = -40.0
SLAB_K = 2                 # k-tiles per bias slab load

F32 = mybir.dt.float32
F32R = mybir.dt.float32r
BF16 = mybir.dt.bfloat16

_CACHE = {}


def build_nc():
    nc = bacc.Bacc("TRN2", target_bir_lowering=False, debug=False,
                   num_devices=NCORES)

    d_xhi = nc.dram_tensor("xhi", [B, S, D], BF16, kind="ExternalInput").ap()
    d_xlo = nc.dram_tensor("xlo", [B, S, D], BF16, kind="ExternalInput").ap()
    d_wq = nc.dram_tensor("wq", [D, HPC * HD], F32R, kind="ExternalInput").ap()
    d_wk = nc.dram_tensor("wk", [D, HPC * HD], F32R, kind="ExternalInput").ap()
    d_wv = nc.dram_tensor("wv", [D, HPC * HD], F32R, kind="ExternalInput").ap()
    d_wo = nc.dram_tensor("wo", [HPC * HD, D], F32R, kind="ExternalInput").ap()
    d_bias = nc.dram_tensor("biasT", [HPC, S, S], BF16, kind="ExternalInput").ap()
    d_wr = nc.dram_tensor("wr", [B, S], F32R, kind="ExternalInput").ap()
    d_w32 = nc.dram_tensor("w32", [B, S], F32, kind="ExternalInput").ap()
    d_cs = nc.dram_tensor("cs", [128, S], F32, kind="ExternalInput").ap()
    d_y = nc.dram_tensor("y", [B, S, D], F32, kind="ExternalOutput").ap()

    with tile.TileContext(nc) as tc, ExitStack() as ctx:
        consts = ctx.enter_context(tc.tile_pool(name="consts", bufs=1))
        wpool = ctx.enter_context(tc.tile_pool(name="wpool", bufs=1))
        qkv = ctx.enter_context(tc.tile_pool(name="qkv", bufs=1))
        panels = ctx.enter_context(tc.tile_pool(name="panels", bufs=2))
        hilo = ctx.enter_context(tc.tile_pool(name="hilo", bufs=2))
        hilo1 = ctx.enter_context(tc.tile_pool(name="hilo1", bufs=1))
        rope = ctx.enter_context(tc.tile_pool(name="rope", bufs=1))
        slabs = ctx.enter_context(tc.tile_pool(name="slabs", bufs=3))
        expp = ctx.enter_context(tc.tile_pool(name="expp", bufs=4))
        normp = ctx.enter_context(tc.tile_pool(name="normp", bufs=1))
        outp = ctx.enter_context(tc.tile_pool(name="outp", bufs=4))
        psum = ctx.enter_context(tc.tile_pool(name="psum", bufs=3, space="PSUM"))
        psum_pv = ctx.enter_context(tc.tile_pool(name="psum_pv", bufs=3, space="PSUM"))
        psum_l = ctx.enter_context(tc.tile_pool(name="psum_l", bufs=2, space="PSUM"))

        def emit_panel(b, sb_i):
            blk = slice(sb_i * SB, sb_i * SB + SB)
            panel = panels.tile([128, DT, SB], F32R, tag="panel", name="panel")
            thi = hilo.tile([128, DT, SB], BF16, tag="thi", name="thi")
            tlo = hilo1.tile([128, DT, SB], BF16, tag="tlo", name="tlo")
            nc.sync.dma_start_transpose(thi, d_xhi[b, blk, :])
            nc.sync.dma_start_transpose(tlo, d_xlo[b, blk, :])
            half = DT // 2
            nc.vector.tensor_add(
                panel[:, :half, :], thi[:, :half, :], tlo[:, :half, :])
            nc.gpsimd.tensor_add(
                panel[:, half:, :], thi[:, half:, :], tlo[:, half:, :])
            return panel

        panel_cache = {}

        # ---------------- constants (emission order = priority) ----------------
        t_w = {}
        for name, dram in (("wq", d_wq), ("wk", d_wk), ("wv", d_wv)):
            t = wpool.tile([128, DT, HPC * HD], F32R, tag=name, name=name)
            nc.gpsimd.dma_start(t, dram.rearrange("(t p) c -> p t c", p=128))
            t_w[name] = t
        t_cs = consts.tile([128, S], F32, tag="cs")
        nc.gpsimd.dma_start(t_cs, d_cs)
        ident = consts.tile([128, 128], F32, tag="ident")
        make_identity(nc, ident)
        ident_bf = consts.tile([128, 128], BF16, tag="ident_bf")
        make_identity(nc, ident_bf)
        t_w32 = consts.tile([128, B, ST], F32, tag="w32")
        nc.gpsimd.dma_start(t_w32, d_w32.rearrange("b (t p) -> p b t", p=128))
        t_wr = consts.tile([128, B, ST], F32R, tag="wr")
        nc.gpsimd.dma_start(t_wr, d_wr.rearrange("b (t p) -> p b t", p=128))
        t_wo = consts.tile([128, HPC, D], F32R, tag="wo")
        nc.gpsimd.dma_start(t_wo, d_wo.rearrange("(h p) c -> p h c", p=128))

        def emit_c_unit(attnT_ref, b_ref, st, nb):
            ss = slice(st * 128, (st + 1) * 128)
            ns = slice(nb * 512, (nb + 1) * 512)
            ops = psum_pv.tile([128, 512], F32, tag="pv", name="ops")
            for h in range(HPC):
                nc.tensor.matmul(
                    ops, attnT_ref[h][:, ss], t_wo[:, h, ns],
                    start=(h == 0), stop=(h == HPC - 1))
            ob = outp.tile([128, 512], F32, tag="ob")
            if nb % 2 == 0:
                nc.scalar.copy(ob, ops)
            else:
                nc.vector.tensor_copy(ob, ops)
            nc.scalar.dma_start(d_y[b_ref, ss, ns], ob)

        pending_c = []
        for b in range(B):
            # ============ phase A: x^T panels -> q^T,k^T (RoPE), v ============
            qT = {}
            kT = {}
            vv = {}
            for h in range(HPC):
                qT[h] = qkv.tile([128, S], F32R, tag=f"qT{h}", name=f"qT{h}")
                kT[h] = qkv.tile([128, S], F32R, tag=f"kT{h}", name=f"kT{h}")
                vv[h] = qkv.tile([128, ST, HD], F32R, tag=f"v{h}", name=f"v{h}")

            for sb_i in range(NSB):
                s0 = sb_i * SB
                blk = slice(s0, s0 + SB)
                if (b, sb_i) in panel_cache:
                    panel = panel_cache.pop((b, sb_i))
                else:
                    panel = emit_panel(b, sb_i)

                for h in range(HPC):
                    hc = slice(h * HD, (h + 1) * HD)
                    # --- q^T and k^T with fused RoPE ---
                    for name, dest in (("wq", qT[h]), ("wk", kT[h])):
                        ps = psum.tile([128, SB], F32, tag="mm")
                        for dt in range(DT):
                            nc.tensor.matmul(
                                ps, t_w[name][:, dt, hc], panel[:, dt, :],
                                start=(dt == 0), stop=(dt == DT - 1))
                        t1 = rope.tile([128, SB], F32, tag="t1")
                        t2 = rope.tile([128, SB], F32, tag="t2")
                        # cs rows 0-63 = sinT, rows 64-127 = cosT
                        nc.vector.tensor_mul(
                            t1[0:64, :], ps[0:64, :], t_cs[64:128, blk])
                        nc.vector.tensor_mul(
                            t1[64:128, :], ps[64:128, :], t_cs[64:128, blk])
                        nc.vector.tensor_mul(
                            t2[0:64, :], ps[64:128, :], t_cs[0:64, blk])
                        nc.vector.tensor_mul(
                            t2[64:128, :], ps[0:64, :], t_cs[0:64, blk])
                        # dest = [x1*c - x2*s ; x2*c + x1*s]
                        nc.gpsimd.tensor_sub(
                            dest[0:64, blk], t1[0:64, :], t2[0:64, :])
                        nc.gpsimd.tensor_add(
                            dest[64:128, blk], t1[64:128, :], t2[64:128, :])
                    # --- v (natural layout) via PE transpose of v^T ---
                    ps = psum.tile([128, SB], F32, tag="mm")
                    for dt in range(DT):
                        nc.tensor.matmul(
                            ps, t_w["wv"][:, dt, hc], panel[:, dt, :],
                            start=(dt == 0), stop=(dt == DT - 1))
                    svt = normp.tile([128, SB], F32, tag="svt")
                    nc.scalar.copy(svt, ps)
                    vch = psum.tile([128, SB // 128, 128], F32, tag="mm")
                    for c4 in range(SB // 128):
                        nc.tensor.transpose(
                            vch[:, c4, :], svt[:, c4 * 128:(c4 + 1) * 128], ident)
                    for c4 in range(SB // 128):
                        stile = (s0 // 128) + c4
                        nc.vector.tensor_scalar_mul(
                            vv[h][:, stile, :], vch[:, c4, :],
                            t_w32[:, b, stile:stile + 1])
                    # drain carried output units from the previous batch
                    if pending_c:
                        emit_c_unit(*pending_c.pop(0))

            # ====== phases B+C software-pipelined over q-blocks ======
            attnT = qT  # norm(j,h) overwrites qT[h][:, qs] after its last read
            for j in range(NQB):
                qs = slice(j * QB, (j + 1) * QB)
                nk = 4 * (j + 1)          # causal: k-tiles 0..nk-1
                steps_left = HPC * nk
                for h in range(HPC):
                    pv = psum_pv.tile([128, QB], F32, tag="pv")
                    lps = psum_l.tile([1, QB], F32, tag="l", name="lps")
                    for g in range((nk + SLAB_K - 1) // SLAB_K):
                        n = min(SLAB_K, nk - g * SLAB_K)
                        slab = slabs.tile([128, SLAB_K, QB], BF16, tag="slab")
                        k0 = g * SLAB_K * 128
                        slab_eng = nc.gpsimd if g % 2 == 0 else nc.scalar
                        slab_eng.dma_start(
                            slab[:, :n, :],
                            d_bias[h, k0:k0 + n * 128, qs].rearrange(
                                "(m p) q -> p m q", p=128))
                        for ml in range(n):
                            m = g * SLAB_K + ml
                            sc = psum.tile([128, QB], F32, tag="mm")
                            nc.tensor.matmul(
                                sc, kT[h][:, m * 128:(m + 1) * 128], qT[h][:, qs],
                                start=True, stop=False)
                            nc.tensor.matmul(
                                sc, ident_bf, slab[:, ml, :],
                                start=False, stop=True, skip_group_check=True)
                            ex = expp.tile([128, QB], F32R, tag="ex")
                            nc.scalar.activation(
                                ex, sc, mybir.ActivationFunctionType.Exp)
                            nc.tensor.matmul(
                                pv, vv[h][:, m, :], ex,
                                start=(m == 0), stop=(m == nk - 1),
                                skip_group_check=True)
                            nc.tensor.matmul(
                                lps, t_wr[:, b, m:m + 1], ex,
                                start=(m == 0), stop=(m == nk - 1),
                                skip_group_check=True)
                            # interleave pending output-projection units
                            if pending_c and (steps_left <= len(pending_c)
                                              or (m + h) % 2 == 0):
                                emit_c_unit(*pending_c.pop(0))
                            steps_left -= 1
                    rl = normp.tile([1, QB], F32, tag="rl")
                    nc.vector.reciprocal(rl, lps)
                    rb = normp.tile([128, QB], F32, tag="rb")
                    nc.gpsimd.partition_broadcast(rb, rl)
                    nc.vector.tensor_mul(attnT[h][:, qs], pv, rb)
                if j < NQB - 1:
                    for c in pending_c:
                        emit_c_unit(*c)
                    pending_c = []
                pending_c = pending_c + [
                    (attnT, b, st, nb) for st in range(4 * j, 4 * j + 4)
                    for nb in range(D // 512)]
            for c in pending_c:
                emit_c_unit(*c)
            pending_c = []

    nc.compile()
    return nc


def _host_prep(x, Wq, Wk, Wv, Wo, policy_mask, memory_weights):
    """Build the per-core input maps."""
    bf = ml_dtypes.bfloat16
    xhi = x.astype(bf)
    xlo = (x.astype(np.float32) - xhi.astype(np.float32)).astype(bf)

    # RoPE tables, transposed: cos2 = [cosT; cosT], sinpm = [-sinT; sinT]
    inv_freq = (1.0 / (ROPE_BASE ** (np.arange(0, HD, 2, dtype=np.float32) / HD)))
    t = np.arange(S, dtype=np.float32)
    freqs = np.outer(t, inv_freq).astype(np.float32)      # [S, 64]
    cosT = np.cos(freqs).T.astype(np.float32)             # [64, S]
    sinT = np.sin(freqs).T.astype(np.float32)
    cs = np.ascontiguousarray(np.concatenate([sinT, cosT], axis=0))

    # memory multiplier w = 1 + GS*mw + 1e-8  (exp(log1p(z)) = 1+z)
    mw = memory_weights.reshape(B, S).astype(np.float64)
    w = (1.0 + GS * mw + 1e-8).astype(np.float32)

    # transposed, causal-masked, pre-scaled policy bias per head (bf16)
    maskT = np.tril(np.full((S, S), MASK_NEG, dtype=np.float32), -1)
    pol = np.asarray(policy_mask, dtype=np.float32)[0]    # [H, S, S]

    in_maps = []
    for c in range(NCORES):
        cols = slice(c * HPC * HD, (c + 1) * HPC * HD)
        bias_c = np.empty((HPC, S, S), dtype=bf)
        for hl in range(HPC):
            hg = c * HPC + hl
            bias_c[hl] = (GS * pol[hg].T + maskT).astype(bf)
        in_maps.append({
            "xhi": xhi, "xlo": xlo,
            "wq": np.ascontiguousarray(Wq[:, cols]),
            "wk": np.ascontiguousarray(Wk[:, cols] * np.float32(SCALE)),
            "wv": np.ascontiguousarray(Wv[:, cols]),
            "wo": np.ascontiguousarray(Wo[cols, :]),
            "biasT": bias_c,
            "wr": w, "w32": w,
            "cs": cs,
        })
    return in_maps


def kernel(x, Wq, Wk, Wv, Wo, bo, policy_mask, memory_weights):
    x = np.asarray(x, dtype=np.float32)
    Wq = np.asarray(Wq, dtype=np.float32)
    Wk = np.asarray(Wk, dtype=np.float32)
    Wv = np.asarray(Wv, dtype=np.float32)
    Wo = np.asarray(Wo, dtype=np.float32)
    bo = np.asarray(bo, dtype=np.float32)

    if "nc" not in _CACHE:
        _CACHE["nc"] = build_nc()
    nc = _CACHE["nc"]

    in_maps = _host_prep(x, Wq, Wk, Wv, Wo, policy_mask, memory_weights)
    res = run_bass_kernel_spmd(nc, in_maps, core_ids=list(range(NCORES)))

    acc = np.zeros((B, S, D), dtype=np.float64)
    for c in range(NCORES):
        acc += res.results[c]["y"].astype(np.float64)
    return (acc + bo.astype(np.float64)).astype(np.float32)


# revision 52
# speedup vs baseline: 1.0062x; 1.0062x over previous
"""EnhancedGovernanceAttention Trainium2 kernel (8 NeuronCores, SPMD).

Sharding: core c owns heads {2c, 2c+1} for BOTH batches (policy_mask is
per-head and batch-shared, so each policy slice is loaded once per core).
Each core computes its heads' attention and a row-parallel partial of the
Wo projection; the host sums the 8 partials and adds bo.

Math notes (vs the jax reference):
 - softmax max-subtraction is dropped: scores ~ N(0,1) + bias in [0,0.2],
   so exp() cannot overflow in fp32; softmax is shift-invariant.
 - log1p memory bias: softmax(s + log(w)) == (w * exp(s)) / sum(w * exp(s))
   with w = 1 + GS*mw + 1e-8, so w is folded into V rows and into the
   denominator matmul -- no per-score log bias needed.
 - causal mask: only lower-triangle k-tiles are computed; the intra-tile
   diagonal mask is baked into the (bf16) policy bias as -40.
 - scores are computed TRANSPOSED ([k, q]) so the PV matmul directly
   yields attn^T, which is the lhsT the output projection needs.
 - x^T is produced by bf16 hi/lo DMA-transposes + one DVE add (exact to
   ~2^-16 relative), avoiding PE-transpose traffic for x.
 - matmuls run in float32r (~1.8e-4 quantization, 4x faster than fp32).
"""

import numpy as np
import ml_dtypes
from contextlib import ExitStack

import concourse.bass as bass
import concourse.tile as tile
from concourse import bacc, mybir
from concourse.bass_utils import run_bass_kernel_spmd
from concourse.masks import make_identity

B, S, D, H, HD = 2, 2048, 2048, 16, 128
GS = 0.1
ROPE_BASE = 10000.0
NCORES = 8
HPC = H // NCORES          # heads per core = 2
SCALE = float(HD) ** -0.5
DT = D // 128              # 16 d-tiles
ST = S // 128              # 16 s-tiles (also k-tiles)
QB = 512                   # q-block width (phase B)
NQB = S // QB              # 4 q-blocks
SB = 256                   # s-block width (phase A panels)
NSB = S // SB              # 8 s-blocks
MASK_NEG = -40.0
SLAB_K = 2                 # k-tiles per bias slab load

F32 = mybir.dt.float32
F32R = mybir.dt.float32r
BF16 = mybir.dt.bfloat16

_CACHE = {}


def build_nc():
    nc = bacc.Bacc("TRN2", target_bir_lowering=False, debug=False,
                   num_devices=NCORES)

    d_xhi = nc.dram_tensor("xhi", [B, S, D], BF16, kind="ExternalInput").ap()
    d_xlo = nc.dram_tensor("xlo", [B, S, D], BF16, kind="ExternalInput").ap()
    d_wq = nc.dram_tensor("wq", [D, HPC * HD], F32R, kind="ExternalInput").ap()
    d_wk = nc.dram_tensor("wk", [D, HPC * HD], F32R, kind="ExternalInput").ap()
    d_wv = nc.dram_tensor("wv", [D, HPC * HD], F32R, kind="ExternalInput").ap()
    d_wo = nc.dram_tensor("wo", [HPC * HD, D], F32R, kind="ExternalInput").ap()
    d_bias = nc.dram_tensor("biasT", [HPC, S, S], BF16, kind="ExternalInput").ap()
    d_wr = nc.dram_tensor("wr", [B, S], F32R, kind="ExternalInput").ap()
    d_w32 = nc.dram_tensor("w32", [B, S], F32, kind="ExternalInput").ap()
    d_cs = nc.dram_tensor("cs", [128, S], F32, kind="ExternalInput").ap()
    d_y = nc.dram_tensor("y", [B, S, D], F32, kind="ExternalOutput").ap()

    with tile.TileContext(nc) as tc, ExitStack() as ctx:
        consts = ctx.enter_context(tc.tile_pool(name="consts", bufs=1))
        wpool = ctx.enter_context(tc.tile_pool(name="wpool", bufs=1))
        qkv = ctx.enter_context(tc.tile_pool(name="qkv", bufs=1))
        panels = ctx.enter_context(tc.tile_pool(name="panels", bufs=2))
        hilo = ctx.enter_context(tc.tile_pool(name="hilo", bufs=2))
        hilo1 = ctx.enter_context(tc.tile_pool(name="hilo1", bufs=1))
        rope = ctx.enter_context(tc.tile_pool(name="rope", bufs=1))
        slabs = ctx.enter_context(tc.tile_pool(name="slabs", bufs=3))
        expp = ctx.enter_context(tc.tile_pool(name="expp", bufs=4))
        normp = ctx.enter_context(tc.tile_pool(name="normp", bufs=1))
        outp = ctx.enter_context(tc.tile_pool(name="outp", bufs=4))
        psum = ctx.enter_context(tc.tile_pool(name="psum", bufs=3, space="PSUM"))
        psum_pv = ctx.enter_context(tc.tile_pool(name="psum_pv", bufs=3, space="PSUM"))
        psum_l = ctx.enter_context(tc.tile_pool(name="psum_l", bufs=2, space="PSUM"))

        def emit_panel(b, sb_i):
            blk = slice(sb_i * SB, sb_i * SB + SB)
            panel = panels.tile([128, DT, SB], F32R, tag="panel", name="panel")
            thi = hilo.tile([128, DT, SB], BF16, tag="thi", name="thi")
            tlo = hilo1.tile([128, DT, SB], BF16, tag="tlo", name="tlo")
            nc.sync.dma_start_transpose(thi, d_xhi[b, blk, :])
            nc.sync.dma_start_transpose(tlo, d_xlo[b, blk, :])
            half = DT // 2
            nc.vector.tensor_add(
                panel[:, :half, :], thi[:, :half, :], tlo[:, :half, :])
            nc.gpsimd.tensor_add(
                panel[:, half:, :], thi[:, half:, :], tlo[:, half:, :])
            return panel

        panel_cache = {}

        # ---------------- constants (emission order = priority) ----------------
        t_w = {}
        for name, dram in (("wq", d_wq), ("wk", d_wk), ("wv", d_wv)):
            t = wpool.tile([128, DT, HPC * HD], F32R, tag=name, name=name)
            nc.gpsimd.dma_start(t, dram.rearrange("(t p) c -> p t c", p=128))
            t_w[name] = t
        t_cs = consts.tile([128, S], F32, tag="cs")
        nc.gpsimd.dma_start(t_cs, d_cs)
        ident = consts.tile([128, 128], F32, tag="ident")
        make_identity(nc, ident)
        ident_bf = consts.tile([128, 128], BF16, tag="ident_bf")
        make_identity(nc, ident_bf)
        t_w32 = consts.tile([128, B, ST], F32, tag="w32")
        nc.gpsimd.dma_start(t_w32, d_w32.rearrange("b (t p) -> p b t", p=128))
        t_wr = consts.tile([128, B, ST], F32R, tag="wr")
        nc.gpsimd.dma_start(t_wr, d_wr.rearrange("b (t p) -> p b t", p=128))
        t_wo = consts.tile([128, HPC, D], F32R, tag="wo")
        nc.gpsimd.dma_start(t_wo, d_wo.rearrange("(h p) c -> p h c", p=128))

        def emit_c_unit(attnT_ref, b_ref, st, nb):
            ss = slice(st * 128, (st + 1) * 128)
            ns = slice(nb * 512, (nb + 1) * 512)
            ops = psum_pv.tile([128, 512], F32, tag="pv", name="ops")
            for h in range(HPC):
                nc.tensor.matmul(
                    ops, attnT_ref[h][:, ss], t_wo[:, h, ns],
                    start=(h == 0), stop=(h == HPC - 1))
            ob = outp.tile([128, 512], F32, tag="ob")
            if nb % 2 == 0:
                nc.scalar.copy(ob, ops)
            else:
                nc.vector.tensor_copy(ob, ops)
            nc.scalar.dma_start(d_y[b_ref, ss, ns], ob)

        pending_c = []
        for b in range(B):
            # ============ phase A: x^T panels -> q^T,k^T (RoPE), v ============
            qT = {}
            kT = {}
            vv = {}
            for h in range(HPC):
                qT[h] = qkv.tile([128, S], F32R, tag=f"qT{h}", name=f"qT{h}")
                kT[h] = qkv.tile([128, S], F32R, tag=f"kT{h}", name=f"kT{h}")
                vv[h] = qkv.tile([128, ST, HD], F32R, tag=f"v{h}", name=f"v{h}")

            for sb_i in range(NSB):
                s0 = sb_i * SB
                blk = slice(s0, s0 + SB)
                if (b, sb_i) in panel_cache:
                    panel = panel_cache.pop((b, sb_i))
                else:
                    panel = emit_panel(b, sb_i)

                for h in range(HPC):
                    hc = slice(h * HD, (h + 1) * HD)
                    # --- q^T and k^T with fused RoPE ---
                    for name, dest in (("wq", qT[h]), ("wk", kT[h])):
                        ps = psum.tile([128, SB], F32, tag="mm")
                        for dt in range(DT):
                            nc.tensor.matmul(
                                ps, t_w[name][:, dt, hc], panel[:, dt, :],
                                start=(dt == 0), stop=(dt == DT - 1))
                        t1 = rope.tile([128, SB], F32, tag="t1")
                        t2 = rope.tile([128, SB], F32, tag="t2")
                        # cs rows 0-63 = sinT, rows 64-127 = cosT
                        nc.vector.tensor_mul(
                            t1[0:64, :], ps[0:64, :], t_cs[64:128, blk])
                        nc.vector.tensor_mul(
                            t1[64:128, :], ps[64:128, :], t_cs[64:128, blk])
                        nc.vector.tensor_mul(
                            t2[0:64, :], ps[64:128, :], t_cs[0:64, blk])
                        nc.vector.tensor_mul(
                            t2[64:128, :], ps[0:64, :], t_cs[0:64, blk])
                        # dest = [x1*c - x2*s ; x2*c + x1*s]
                        nc.gpsimd.tensor_sub(
                            dest[0:64, blk], t1[0:64, :], t2[0:64, :])
                        nc.gpsimd.tensor_add(
                            dest[64:128, blk], t1[64:128, :], t2[64:128, :])
                    # --- v (natural layout) via PE transpose of v^T ---
                    ps = psum.tile([128, SB], F32, tag="mm")
                    for dt in range(DT):
                        nc.tensor.matmul(
                            ps, t_w["wv"][:, dt, hc], panel[:, dt, :],
                            start=(dt == 0), stop=(dt == DT - 1))
                    svt = normp.tile([128, SB], F32, tag="svt")
                    nc.scalar.copy(svt, ps)
                    vch = psum.tile([128, SB // 128, 128], F32, tag="mm")
                    for c4 in range(SB // 128):
                        nc.tensor.transpose(
                            vch[:, c4, :], svt[:, c4 * 128:(c4 + 1) * 128], ident)
                    for c4 in range(SB // 128):
                        stile = (s0 // 128) + c4
                        nc.vector.tensor_scalar_mul(
                            vv[h][:, stile, :], vch[:, c4, :],
                            t_w32[:, b, stile:stile + 1])
                    # drain carried output units from the previous batch
                    if pending_c:
                        emit_c_unit(*pending_c.pop(0))

            # ====== phases B+C software-pipelined over q-blocks ======
            attnT = qT  # norm(j,h) overwrites qT[h][:, qs] after its last read
            for j in range(NQB):
                qs = slice(j * QB, (j + 1) * QB)
                nk = 4 * (j + 1)          # causal: k-tiles 0..nk-1
                steps_left = HPC * nk
                for h in range(HPC):
                    pv = psum_pv.tile([128, QB], F32, tag="pv")
                    lps = psum_l.tile([1, QB], F32, tag="l", name="lps")
                    for g in range((nk + SLAB_K - 1) // SLAB_K):
                        n = min(SLAB_K, nk - g * SLAB_K)
                        slab = slabs.tile([128, SLAB_K, QB], BF16, tag="slab")
                        k0 = g * SLAB_K * 128
                        slab_eng = nc.gpsimd if g % 2 == 0 else nc.scalar
                        slab_eng.dma_start(
                            slab[:, :n, :],
                            d_bias[h, k0:k0 + n * 128, qs].rearrange(
                                "(m p) q -> p m q", p=128))
                        for ml in range(n):
                            m = g * SLAB_K + ml
                            # columns q < 128*m are fully causal-masked; skip
                            # them, but keep N >= 256 (f32r speed) when useful
                            off = max(0, (m - 4 * j) * 128)
                            qso = slice(j * QB + off, (j + 1) * QB)
                            sc = psum.tile([128, QB], F32, tag="mm")
                            nc.tensor.matmul(
                                sc[:, off:], kT[h][:, m * 128:(m + 1) * 128],
                                qT[h][:, qso],
                                start=True, stop=False)
                            nc.tensor.matmul(
                                sc[:, off:], ident_bf, slab[:, ml, off:],
                                start=False, stop=True, skip_group_check=True)
                            ex = expp.tile([128, QB], F32R, tag="ex")
                            nc.scalar.activation(
                                ex[:, off:], sc[:, off:],
                                mybir.ActivationFunctionType.Exp)
                            nc.tensor.matmul(
                                pv[:, off:], vv[h][:, m, :], ex[:, off:],
                                start=(m == 0), stop=(m == nk - 1),
                                skip_group_check=True)
                            nc.tensor.matmul(
                                lps[:, off:], t_wr[:, b, m:m + 1], ex[:, off:],
                                start=(m == 0), stop=(m == nk - 1),
                                skip_group_check=True)
                            # interleave pending output-projection units
                            if pending_c and (steps_left <= len(pending_c)
                                              or (m + h) % 2 == 0):
                                emit_c_unit(*pending_c.pop(0))
                            steps_left -= 1
                    rl = normp.tile([1, QB], F32, tag="rl")
                    nc.vector.reciprocal(rl, lps)
                    rb = normp.tile([128, QB], F32, tag="rb")
                    nc.gpsimd.partition_broadcast(rb, rl)
                    nc.vector.tensor_mul(attnT[h][:, qs], pv, rb)
                if j < NQB - 1:
                    for c in pending_c:
                        emit_c_unit(*c)
                    pending_c = []
                pending_c = pending_c + [
                    (attnT, b, st, nb) for st in range(4 * j, 4 * j + 4)
                    for nb in range(D // 512)]
            for c in pending_c:
                emit_c_unit(*c)
            pending_c = []

    nc.compile()
    return nc


def _host_prep(x, Wq, Wk, Wv, Wo, policy_mask, memory_weights):
    """Build the per-core input maps."""
    bf = ml_dtypes.bfloat16
    xhi = x.astype(bf)
    xlo = (x.astype(np.float32) - xhi.astype(np.float32)).astype(bf)

    # RoPE tables, transposed: cos2 = [cosT; cosT], sinpm = [-sinT; sinT]
    inv_freq = (1.0 / (ROPE_BASE ** (np.arange(0, HD, 2, dtype=np.float32) / HD)))
    t = np.arange(S, dtype=np.float32)
    freqs = np.outer(t, inv_freq).astype(np.float32)      # [S, 64]
    cosT = np.cos(freqs).T.astype(np.float32)             # [64, S]
    sinT = np.sin(freqs).T.astype(np.float32)
    cs = np.ascontiguousarray(np.concatenate([sinT, cosT], axis=0))

    # memory multiplier w = 1 + GS*mw + 1e-8  (exp(log1p(z)) = 1+z)
    mw = memory_weights.reshape(B, S).astype(np.float64)
    w = (1.0 + GS * mw + 1e-8).astype(np.float32)

    # transposed, causal-masked, pre-scaled policy bias per head (bf16)
    maskT = np.tril(np.full((S, S), MASK_NEG, dtype=np.float32), -1)
    pol = np.asarray(policy_mask, dtype=np.float32)[0]    # [H, S, S]

    in_maps = []
    for c in range(NCORES):
        cols = slice(c * HPC * HD, (c + 1) * HPC * HD)
        bias_c = np.empty((HPC, S, S), dtype=bf)
        for hl in range(HPC):
            hg = c * HPC + hl
            bias_c[hl] = (GS * pol[hg].T + maskT).astype(bf)
        in_maps.append({
            "xhi": xhi, "xlo": xlo,
            "wq": np.ascontiguousarray(Wq[:, cols]),
            "wk": np.ascontiguousarray(Wk[:, cols] * np.float32(SCALE)),
            "wv": np.ascontiguousarray(Wv[:, cols]),
            "wo": np.ascontiguousarray(Wo[cols, :]),
            "biasT": bias_c,
            "wr": w, "w32": w,
            "cs": cs,
        })
    return in_maps


def kernel(x, Wq, Wk, Wv, Wo, bo, policy_mask, memory_weights):
    x = np.asarray(x, dtype=np.float32)
    Wq = np.asarray(Wq, dtype=np.float32)
    Wk = np.asarray(Wk, dtype=np.float32)
    Wv = np.asarray(Wv, dtype=np.float32)
    Wo = np.asarray(Wo, dtype=np.float32)
    bo = np.asarray(bo, dtype=np.float32)

    if "nc" not in _CACHE:
        _CACHE["nc"] = build_nc()
    nc = _CACHE["nc"]

    in_maps = _host_prep(x, Wq, Wk, Wv, Wo, policy_mask, memory_weights)
    res = run_bass_kernel_spmd(nc, in_maps, core_ids=list(range(NCORES)))

    acc = np.zeros((B, S, D), dtype=np.float64)
    for c in range(NCORES):
        acc += res.results[c]["y"].astype(np.float64)
    return (acc + bo.astype(np.float64)).astype(np.float32)
